# revision 4
# baseline (speedup 1.0000x reference)
"""Trainium2 Bass kernel for nn_ApplyAttentionPolicyMap.

Reference computes out = concat(logits, pp_logits) @ fc1 where fc1 is a
4288x1858 one-hot column-selection map: out[b, j] = flat[b, sel[j]].

Strategy (8 NeuronCores, sharded over output columns by source row):
  * Host: lay the activations feature-major (xT [4288, 8192]) in bf16 so the
    selection becomes a row gather at half the HBM traffic (the policy map
    only moves data, so bf16 rounding bounds the relative error at 2^-9).
    Sort the 1858 output columns by their source row sel[j] and split them
    into 8 equal groups; core k receives the contiguous band of xT rows
    covering its group (about 1/8th of the input) plus the group's local
    row indices.  Fat gather rows keep the SWDGE descriptor count tiny
    (the ~1.4us per-indirect-DMA cadence is what limited a batch-sharded
    variant, which needed 15 instructions on the critical path).
  * Device: idx load, then 8 pipelined indirect row-gathers HBM->SBUF
    (2 row chunks x 4 batch quarters, ~0.5MB each), each chased by a direct
    HWDGE store of the gathered rows to the feature-major output shard.
    Sync and Scalar split the stores so both HWDGE rings run, overlapping
    the write leg with the ongoing gather reads.
  * Host again: un-permute columns, restore batch-major layout and f32.

The NEFF epilogue walrus emits clears every physical semaphore one
instruction at a time (~6us for the default 256); a wrapper around the
walrus driver passes --max-sem-num to shrink that sweep.
"""

import os
import stat
import tempfile

import numpy as np
import ml_dtypes

import concourse.bacc as bacc
import concourse.bass as bass
import concourse.bass_utils as bass_utils
import concourse.mybir as mybir
from concourse.bass_utils import run_bass_kernel_spmd

N_CORES = 8
B = 8192
NQ = 4                            # batch quarters for gather/store pipelining
B_Q = B // NQ                     # 2048
IN_DIM = 64 * 64 + 8 * 24         # 4288
OUT_DIM = 1858
NCOL = (OUT_DIM + N_CORES - 1) // N_CORES  # 233 columns per core (padded)
NR = (128, NCOL - 128)            # rows per gather chunk (128, 105)

_DT = mybir.dt.bfloat16

_cached = {}


def _install_walrus_shim():
    """Cap walrus's physical semaphore pool so the NEFF epilogue's
    per-semaphore clear sweep shrinks from ~256 to ~48 instructions."""
    if _cached.get("shim"):
        return
    try:
        real = bass_utils.get_walrus_driver()
        fd, path = tempfile.mkstemp(prefix="walrus_shim_", suffix=".sh")
        with os.fdopen(fd, "w") as f:
            f.write(f'#!/bin/sh\nexec "{real}" "$@" --max-sem-num=48\n')
        os.chmod(path, os.stat(path).st_mode | stat.S_IEXEC)
        bass_utils.get_walrus_driver = lambda: path
    except Exception:
        pass
    _cached["shim"] = True


def _build_nc(r_max: int):
    _install_walrus_shim()
    # Keep bass's own semaphore ids inside walrus's shrunken pool.
    orig = bass.get_kernel_semaphore_range
    bass.get_kernel_semaphore_range = lambda: range(
        bass.get_walrus_max_sem_num(), bass.get_walrus_max_sem_num() + 40
    )
    try:
        nc = bacc.Bacc("TRN2")
    finally:
        bass.get_kernel_semaphore_range = orig

    xs = [
        nc.declare_dram_parameter(f"xs{q}", [r_max, B_Q], _DT, isOutput=False)
        for q in range(NQ)
    ]
    idx_d = nc.declare_dram_parameter("idx", [128, 2], mybir.dt.int32, isOutput=False)
    outs_d = [
        nc.declare_dram_parameter(f"out{q}", [NCOL, B_Q], _DT, isOutput=True)
        for q in range(NQ)
    ]

    from contextlib import ExitStack

    # gather issue order: all c0 chunks (big), then all c1 (small), quarters
    # interleaved Sync-side/Scalar-side so both store rings start early
    order = [(0, 0), (2, 0), (1, 0), (3, 0), (0, 1), (2, 1), (1, 1), (3, 1)]

    with (
        nc.sbuf_tensor("gath", [128, NQ, 2, B_Q], _DT) as gath,
        nc.sbuf_tensor("idx_sb", [128, 2], mybir.dt.int32) as idx_sb,
        nc.semaphore("io") as io_sem,
        nc.semaphore("outs") as out_sem,
        nc.semaphore("outs2") as out2_sem,
        ExitStack() as stack,
        nc.Block() as block,
    ):
        gsem = {
            qc: stack.enter_context(nc.semaphore(f"g{qc[0]}{qc[1]}"))  # noqa: ANT232
            for qc in order
        }

        @block.sync
        def _(s):
            # idx load on HWDGE; everything hangs off this ~2.5us round trip.
            s.dma_start(idx_sb[:, :], idx_d[:, :]).then_inc(io_sem, 16)
            # stores for quarters 0-1; completion covered by block-end drain
            for q, c in ((0, 0), (1, 0), (0, 1), (1, 1)):
                nr = NR[c]
                s.wait_ge(gsem[(q, c)], 16)
                s.dma_start(
                    out=outs_d[q][c * 128 : c * 128 + nr, :],
                    in_=gath[0:nr, q, c, :],
                ).then_inc(out_sem, 16)

        @block.gpsimd
        def _(g):
            g.wait_ge(io_sem, 16)
            for q, c in order:
                nr = NR[c]
                g.indirect_dma_start(
                    out=gath[0:nr, q, c, :],
                    out_offset=None,
                    in_=xs[q][:, :],
                    in_offset=bass.IndirectOffsetOnAxis(
                        ap=idx_sb[0:nr, c : c + 1], axis=0
                    ),
                ).then_inc(gsem[(q, c)], 16)

        @block.scalar
        def _(s):
            # stores for quarters 2-3 on the second HWDGE ring
            for q, c in ((2, 0), (3, 0), (2, 1), (3, 1)):
                nr = NR[c]
                s.wait_ge(gsem[(q, c)], 16)
                s.dma_start(
                    out=outs_d[q][c * 128 : c * 128 + nr, :],
                    in_=gath[0:nr, q, c, :],
                ).then_inc(out2_sem, 16)

    nc.compile()
    return nc


def _get_nc(r_max: int):
    if r_max not in _cached:
        _cached[r_max] = _build_nc(r_max)
    return _cached[r_max]


def _extract_sel(fc1: np.ndarray):
    """Return sel[j] with fc1 == one_hot(sel), or None if fc1 is not an
    exact one-hot column-selection map."""
    if fc1.shape != (IN_DIM, OUT_DIM):
        return None
    sel = np.argmax(fc1, axis=0)
    ok = (fc1[sel, np.arange(OUT_DIM)] == 1.0).all()
    if not ok:
        return None
    # each column must have exactly one nonzero
    nnz = np.count_nonzero(fc1, axis=0)
    if not (nnz == 1).all():
        return None
    return sel.astype(np.int64)


def _plan_shards(sel: np.ndarray):
    """Assign output columns to cores by sorted source row.

    Returns (groups, starts, r_max):
      groups[k]: the output-column ids owned by core k (sorted by sel)
      starts[k]: first xT row of core k's contiguous input band
      r_max:     uniform band height (rows) across cores
    """
    order = np.argsort(sel, kind="stable")
    base, rem = divmod(OUT_DIM, N_CORES)
    groups, lo = [], 0
    for k in range(N_CORES):
        n = base + (1 if k < rem else 0)
        groups.append(order[lo : lo + n])
        lo += n
    r_max = 1
    for g in groups:
        rows = sel[g]
        r_max = max(r_max, int(rows.max() - rows.min() + 1))
    starts = []
    for g in groups:
        r0 = int(sel[g].min())
        starts.append(min(r0, IN_DIM - r_max))
    return groups, starts, r_max


def _build_idx_tensor(local_rows: np.ndarray) -> np.ndarray:
    """int32 [128, 2]: idx[p, c] = local_rows[c*128 + p] (0 for padding)."""
    pad = np.zeros(2 * 128, dtype=np.int32)
    pad[: local_rows.shape[0]] = local_rows.astype(np.int32)
    return pad.reshape(2, 128).T.copy()


def kernel(logits: np.ndarray, pp_logits: np.ndarray, fc1: np.ndarray) -> np.ndarray:
    logits = np.asarray(logits, dtype=np.float32)
    pp_logits = np.asarray(pp_logits, dtype=np.float32)
    fc1 = np.asarray(fc1, dtype=np.float32)
    b = logits.shape[0]
    flat = np.concatenate(
        [logits.reshape(b, 64 * 64), pp_logits.reshape(b, 8 * 24)], axis=1
    )

    sel = _extract_sel(fc1)
    if sel is None or b != B:
        # Degenerate input (fc1 not an exact selection map, or unexpected
        # batch) — fall back to the dense reference computation.
        return flat @ fc1

    groups, starts, r_max = _plan_shards(sel)
    nc = _get_nc(r_max)
    xT = np.ascontiguousarray(flat.T.astype(ml_dtypes.bfloat16))  # [4288, 8192]

    in_maps = []
    for k in range(N_CORES):
        r0 = starts[k]
        band = xT[r0 : r0 + r_max]
        m = {
            f"xs{q}": np.ascontiguousarray(band[:, q * B_Q : (q + 1) * B_Q])
            for q in range(NQ)
        }
        m["idx"] = _build_idx_tensor(sel[groups[k]] - r0)
        in_maps.append(m)

    res = run_bass_kernel_spmd(nc, in_maps, list(range(N_CORES)))

    outT = np.empty((OUT_DIM, B), dtype=np.float32)
    for k in range(N_CORES):
        n = groups[k].shape[0]
        for q in range(NQ):
            outT[groups[k], q * B_Q : (q + 1) * B_Q] = (
                res.results[k][f"out{q}"][:n].astype(np.float32)
            )
    return np.ascontiguousarray(outT.T)
